# revision 39
# baseline (speedup 1.0000x reference)
# Trainium2 Bass kernel for nn_Attention: out = softmax(x @ (y@W + b) + mask*-1e9) @ x
# Sharding: data-parallel over batch, 1 batch element per NeuronCore (8 cores).
#
# Per-core math (S = D = 1024), reassociated as (x@y)@W:
#   gT[d,s]  = sum_k x[s,k] y[k,d]                      (PE pass 1)
#   a[s,t]   = sum_d g[s,d] W[d,t] + rowsum_x[s]*b[t]   (PE pass 2 + rank-1 on DVE)
#   p        = softmax(a + mask*-1e9)                   (DVE/Act)
#   out[s,e] = sum_t p[s,t] x[t,e]                      (PE pass 3)
#
# Precision: plain fp16 single-pass matmuls (fp16 products accumulate exactly
# in fp32 PSUM). End-to-end rel err ~2.3e-3 against the fp32 reference (gate
# 2e-2). The host pre-scales and rounds inputs to fp16 (x,y by 16, W by 1024
# -- exactly the values the kernel would produce on-chip), which halves input
# HBM traffic and removes the entire on-chip scale/cast pipeline; a single DGE
# queue only sustains ~200 GB/s, so inputs ride two HW queues in parallel
# (sync: x+mask+stores, scalar: y+W) directly into persistent slabs.
#
# x and the softmax rows are transposed on the PE (the DMA-xbar transpose path
# ran at ~44 GB/s and starved the output stage). 1/(rowsum*SX) is folded into
# the exp rows before transposing, so the output matmul's PSUM holds the final
# result and the epilogue is a plain fp16 copy + store. Dummy matmuls (HAM
# filler) keep PE utilization up during the DMA-paced phase so the clock
# governor stays at 8/8.
import sys

import numpy as np

for _p in ("/opt/trn_rl_repo",):
    if _p not in sys.path:
        sys.path.insert(0, _p)

import concourse.bass as bass
from concourse import bacc
import concourse.mybir as mybir
import concourse.tile as tile
from concourse.bass_utils import run_bass_kernel_spmd

F32 = mybir.dt.float32
F16 = mybir.dt.float16

P = 128
FD = 512  # matmul moving free dim (one fp32 PSUM bank)

SX = 16.0  # x / y pre-scale (applied on host)
SW = 1024.0  # W pre-scale (applied on host)
SLOG = SX * SX * SW  # net logit scale = 2**18
MASKC = -1.0e9 * SLOG

ALU = mybir.AluOpType
ACTF = mybir.ActivationFunctionType
AXIS = mybir.AxisListType


def build_nc(n=1024, with_bias=False):
    """Build the per-core Bass program (SPMD: same program on all 8 cores)."""
    NT = n // P  # 128-tiles per dim
    NH = n // FD  # 512-halves per dim
    HC = NT // NH  # transposed chunks per half (4)

    nc = bacc.Bacc("TRN2", target_bir_lowering=False, debug=False)
    x_d = nc.dram_tensor("x16", [n, n], F16, kind="ExternalInput")
    xt_d = nc.dram_tensor("xT16", [n, n], F16, kind="ExternalInput")
    y_d = nc.dram_tensor("y16", [n, n], F16, kind="ExternalInput")
    mask_d = nc.dram_tensor("mask16", [n, n], F16, kind="ExternalInput")
    w_d = nc.dram_tensor("W16", [n, n], F16, kind="ExternalInput")
    b_d = None
    if with_bias:
        b_d = nc.dram_tensor("bvec", [1, n], F32, kind="ExternalInput")
    id_d = nc.dram_tensor("ident16", [P, P], F16, kind="ExternalInput")
    out_d = nc.dram_tensor("out", [n, n], F16, kind="ExternalOutput")

    with tile.TileContext(nc) as tc:
        import contextlib

        ctx = contextlib.ExitStack()
        with ctx:
            persist = ctx.enter_context(tc.tile_pool(name="persist", bufs=1))
            epi = ctx.enter_context(tc.tile_pool(name="epi", bufs=4))
            ehp = ctx.enter_context(tc.tile_pool(name="ehp", bufs=4))
            small = ctx.enter_context(tc.tile_pool(name="small", bufs=4))
            psum = ctx.enter_context(tc.tile_pool(name="psum", bufs=3, space="PSUM"))
            psum_r = ctx.enter_context(
                tc.tile_pool(name="psum_r", bufs=1, space="PSUM")
            )
            dram = ctx.enter_context(tc.tile_pool(name="dram", bufs=1, space="DRAM"))

            # ---- persistent slabs (fp16 slabs are [P, NT, n] = 16KB/part) ---
            x_hi = persist.tile([P, NT, n], F16, tag="x_hi")
            y_hi = persist.tile([P, NT, n], F16, tag="y_hi")
            gt_hi = persist.tile([P, NT, n], F16, tag="gt_hi")
            w_hi = persist.tile([P, NT, n], F16, tag="w_hi")
            mask_sl = persist.tile([P, NT, n], F16, tag="mask_sl")
            # transposed x, one slab per s-half: [P, kt, hc, P]
            xTh = [
                persist.tile([P, NT, HC, P], F16, tag=f"xTh_{h}", name=f"xTh_{h}")
                for h in range(NH)
            ]
            ident16 = persist.tile([P, P], F16, tag="ident16")
            scratch = persist.tile([P, FD], F16, tag="scratch")
            rxs = None
            if with_bias:
                rxs = persist.tile([P, NT], F32, tag="rxs")
            recip = [
                persist.tile([P, 1], F32, tag=f"recip{i}", name=f"recip{i}")
                for i in range(NT)
            ]
            et = [
                [
                    persist.tile([P, HC, P], F16, tag=f"et{i}_{h}", name=f"et{i}_{h}")
                    for h in range(NH)
                ]
                for i in range(NT)
            ]
            b_bc = b_sb = None
            if with_bias:
                b_bc = persist.tile([P, n], F32, tag="b_bc")
                b_sb = persist.tile([1, n], F32, tag="b_sb")

            # ident loads first on the sync queue; the PE warmup hangs off it
            nc.sync.dma_start(ident16, id_d[:, :])

            # HAM warm-up: dummy matmuls so the PE clock ramps to 8/8 before
            # the first real transpose/matmul arrives.
            nc.gpsimd.memset(scratch, 0.0)
            wps = psum_r.tile([P, FD], F32, tag="rsx", name="warm_ps")
            for i in range(7):
                nc.tensor.matmul(
                    wps[:, 0:P],
                    lhsT=ident16,
                    rhs=ident16,
                    start=(i == 0),
                    stop=(i == 6),
                )

            def ham_fill(k, name):
                hp = psum_r.tile([P, FD], F32, tag="rsx", name=name)
                for i in range(k):
                    nc.tensor.matmul(
                        hp,
                        lhsT=scratch[:, 0:P],
                        rhs=scratch,
                        start=(i == 0),
                        stop=(i == k - 1),
                    )

            # ---- parallel input streams, striped across both HW DGE queues --
            # Strict need-order: (xT half-0 chunk kt, y kt) pairs for the g
            # wavefront, then xT half-1, W, mask, x rows (out stage only).
            # xTh[h][:, c, hc, pp] = xT16[c*P+p, (h*HC+hc)*P+pp]: one [P, FD]
            # block DMA per (h, c), in the g ladder's consumption order.
            qs = [nc.sync, nc.scalar]

            def xt_load(q, h, c):
                q.dma_start(
                    xTh[h][:, c, :, :],
                    xt_d[P * c : P * (c + 1), FD * h : FD * (h + 1)],
                )

            for kt in range(NT):
                xt_load(qs[kt % 2], 0, kt)
                qs[(kt + 1) % 2].dma_start(
                    y_hi[:, kt, :], y_d[P * kt : P * (kt + 1), :]
                )
            for c in range(NT):
                xt_load(qs[c % 2], 1, c)
            for dt in range(NT):
                qs[dt % 2].dma_start(
                    w_hi[:, dt, :], w_d[P * dt : P * (dt + 1), :]
                )
            for st in range(NT):
                qs[st % 2].dma_start(
                    mask_sl[:, st, :], mask_d[P * st : P * (st + 1), :]
                )
            for it in range(NT):
                qs[it % 2].dma_start(x_hi[:, it, :], x_d[P * it : P * (it + 1), :])

            # bias term applied on DVE in the softmax epilogue:
            #   am += (SLOG/SX * b)[t] * rowsum_x16[s]
            # b broadcast across partitions via a DRAM bounce on the SWDGE
            # queue, which is otherwise idle.
            if with_bias:
                nc.gpsimd.dma_start(b_sb, b_d[:, :])
                nc.vector.tensor_scalar_mul(b_sb, b_sb, SLOG / SX)
                b_dr = dram.tile([1, n], F32, name="b_dr")
                nc.gpsimd.dma_start(b_dr[0:1, :], b_sb)
                bsrc = b_dr[0:1, :]
                nc.gpsimd.dma_start(
                    b_bc[:, :],
                    bass.AP(
                        tensor=bsrc.tensor,
                        offset=bsrc.offset,
                        ap=[[0, P], bsrc.ap[1]],
                    ),
                )


            # g-stage wavefront psums: held open across the whole kt ladder so
            # each arriving y tile immediately unlocks matmuls.
            WFD = list(range(HC))  # dt groups 0..3 of g half 0
            g_ps = {}
            for dt in WFD:
                g_ps[dt] = psum.tile(
                    [P, FD], F32, tag="gwf", bufs=4, name=f"gwf_{dt}"
                )

            def g_wf_step(kt):
                for dt in WFD:
                    nc.tensor.matmul(
                        g_ps[dt],
                        lhsT=y_hi[:, kt, P * dt : P * (dt + 1)],
                        rhs=xTh[0][:, kt, :, :],
                        start=(kt == 0),
                        stop=(kt == NT - 1),
                    )

            for kt in range(NT):
                g_wf_step(kt)
                ham_fill(1, f"hamy_{kt}")

            for dt in WFD:
                nc.vector.tensor_copy(gt_hi[:, dt, 0:FD], g_ps[dt])
            del g_ps

            # ---- rest of g: half 0 dt 4..7, then all of half 1 --------------
            def g_group(sh, dt):
                ps = psum.tile([P, FD], F32, tag="mm", name=f"g{sh}_{dt}")
                for kt in range(NT):
                    nc.tensor.matmul(
                        ps,
                        lhsT=y_hi[:, kt, P * dt : P * (dt + 1)],
                        rhs=xTh[sh][:, kt, :, :],
                        start=(kt == 0),
                        stop=(kt == NT - 1),
                    )
                nc.vector.tensor_copy(gt_hi[:, dt, FD * sh : FD * (sh + 1)], ps)

            for dt in range(HC, NT):
                g_group(0, dt)
            for dt in range(NT):
                g_group(1, dt)

            # ---- a stage + softmax, with eT transposes pipelined ------------
            eh_l = [None] * NT
            rs_l = [None] * NT

            def a_emit(st):
                if with_bias:
                    # bias rowsum from the x rows, paced one tile per a-tile
                    # so it never head-of-line blocks the DVE queue
                    nc.vector.tensor_reduce(
                        rxs[:, st : st + 1],
                        x_hi[:, st, :],
                        axis=AXIS.X,
                        op=ALU.add,
                    )
                am = epi.tile([P, n], F32, tag="am", name=f"am{st}")
                for th in range(NH):
                    ps = psum.tile([P, FD], F32, tag="mm", name=f"a{st}_{th}")
                    for dt in range(NT):
                        nc.tensor.matmul(
                            ps,
                            lhsT=gt_hi[:, dt, P * st : P * (st + 1)],
                            rhs=w_hi[:, dt, FD * th : FD * (th + 1)],
                            start=(dt == 0),
                            stop=(dt == NT - 1),
                        )
                    # masked scaled logits: am = mask*MASKC + psum
                    nc.vector.scalar_tensor_tensor(
                        out=am[:, FD * th : FD * (th + 1)],
                        in0=mask_sl[:, st, FD * th : FD * (th + 1)],
                        scalar=MASKC,
                        in1=ps,
                        op0=ALU.mult,
                        op1=ALU.add,
                    )
                if with_bias:
                    # am += b_bc[t] * rowsum_x16[s]  (rank-1 bias)
                    nc.vector.scalar_tensor_tensor(
                        out=am,
                        in0=b_bc,
                        scalar=rxs[:, st : st + 1],
                        in1=am,
                        op0=ALU.mult,
                        op1=ALU.add,
                    )
                nm = small.tile([P, 1], F32, tag="nm", name=f"nm{st}")
                nc.vector.tensor_reduce(
                    nm, am, axis=AXIS.X, op=ALU.max, negate=True
                )
                nms = small.tile([P, 1], F32, tag="nms", name=f"nms{st}")
                nc.vector.tensor_scalar_mul(nms, nm, 1.0 / SLOG)
                eh = ehp.tile([P, n], F16, tag="eh", name=f"eh{st}")
                rs = small.tile([P, 1], F32, tag="rs", name=f"rs{st}")
                nc.scalar.activation(
                    eh, am, ACTF.Exp, bias=nms, scale=1.0 / SLOG, accum_out=rs
                )
                eh_l[st] = eh
                rs_l[st] = rs

            def finish_emit(st):
                # fold 1/(rowsum * SX) into the exp rows so the out-stage psum
                # holds the final output directly
                rs2 = small.tile([P, 1], F32, tag="rs2", name=f"rs2_{st}")
                nc.vector.tensor_scalar_mul(rs2, rs_l[st], SX)
                nc.vector.reciprocal(recip[st], rs2)
                nc.vector.tensor_scalar_mul(eh_l[st], eh_l[st], recip[st])

            def t_emit(st):
                # transpose the scaled exp rows on the PE (8 chunks of 128x128)
                for h in range(NH):
                    pt = psum.tile(
                        [P, HC, P], F16, tag="gwf", bufs=4, name=f"eT{st}_{h}"
                    )
                    for j in range(HC):
                        c = h * HC + j
                        nc.tensor.transpose(
                            pt[:, j, :],
                            eh_l[st][:, P * c : P * (c + 1)],
                            ident16,
                        )
                    nc.vector.tensor_copy(et[st][h], pt)

            def out_emit(st):
                # both halves interleaved: consecutive matmuls share lhsT
                opair = []
                for h in range(NH):
                    ps = psum.tile([P, FD], F32, tag="mm", name=f"o{st}_{h}")
                    for tt in range(NT):
                        nc.tensor.matmul(
                            ps,
                            lhsT=et[st][tt // HC][:, tt % HC, :],
                            rhs=x_hi[:, tt, FD * h : FD * (h + 1)],
                            start=(tt == 0),
                            stop=(tt == NT - 1),
                        )
                    opair.append((h, ps))
                for h, ps in opair:
                    ob = epi.tile([P, FD], F16, tag="ob", name=f"ob{st}_{h}")
                    if h == 0:
                        nc.scalar.copy(ob, ps)
                        nc.sync.dma_start(
                            out_d[P * st : P * (st + 1), 0:FD], ob
                        )
                    else:
                        nc.vector.tensor_copy(ob, ps)
                        nc.scalar.dma_start(
                            out_d[P * st : P * (st + 1), FD:n], ob
                        )

            for st in range(NT):
                a_emit(st)
                if st >= 2:
                    finish_emit(st - 2)
                    t_emit(st - 2)
            finish_emit(NT - 2)
            t_emit(NT - 2)
            out_emit(0)
            finish_emit(NT - 1)
            t_emit(NT - 1)
            for st in range(1, NT):
                out_emit(st)
    nc.compile()
    return nc


_NC_CACHE = {}


def _get_nc(n=1024, with_bias=False):
    key = (n, with_bias)
    if key not in _NC_CACHE:
        _NC_CACHE[key] = build_nc(n, with_bias)
    return _NC_CACHE[key]


def _prep_in_maps(x, y, mask, W, b, with_bias=False):
    """Host-side shard prep: pre-scaled fp16 copies of the inputs."""
    n = x.shape[-1]
    x16 = (np.asarray(x, dtype=np.float32) * SX).astype(np.float16)
    y16 = (np.asarray(y, dtype=np.float32) * SX).astype(np.float16)
    m16 = np.asarray(mask, dtype=np.float32).astype(np.float16)
    W16 = np.ascontiguousarray(
        (np.asarray(W, dtype=np.float32) * SW).astype(np.float16)
    )
    bc = np.ascontiguousarray(np.asarray(b, dtype=np.float32).reshape(1, n))
    idc = np.eye(P, dtype=np.float16)
    in_maps = []
    for c in range(x.shape[0]):
        in_maps.append(
            {
                "x16": np.ascontiguousarray(x16[c]),
                "xT16": np.ascontiguousarray(x16[c].T),
                "y16": np.ascontiguousarray(y16[c]),
                "mask16": np.ascontiguousarray(m16[c]),
                "W16": W16,
                "ident16": idc,
            }
        )
        if with_bias:
            in_maps[-1]["bvec"] = bc
    return in_maps


def kernel(x, y, mask, W, b):
    """Full-input entry point: shard over batch across 8 cores, run, gather."""
    n = x.shape[-1]
    with_bias = bool(np.any(np.asarray(b)))
    nc = _get_nc(n, with_bias)
    in_maps = _prep_in_maps(x, y, mask, W, b, with_bias)
    res = run_bass_kernel_spmd(nc, in_maps, core_ids=list(range(len(in_maps))))
    return np.stack([r["out"] for r in res.results], axis=0).astype(np.float32)


# revision 40
# speedup vs baseline: 1.1762x; 1.1762x over previous
# Trainium2 Bass kernel for nn_Attention: out = softmax(x @ (y@W + b) + mask*-1e9) @ x
# Sharding: data-parallel over batch, 1 batch element per NeuronCore (8 cores).
#
# Per-core math (S = D = 1024), reassociated as (x@y)@W:
#   gT[d,s]  = sum_k x[s,k] y[k,d]                      (PE pass 1)
#   a[s,t]   = sum_d g[s,d] W[d,t] + rowsum_x[s]*b[t]   (PE pass 2 + rank-1 on DVE)
#   p        = softmax(a + mask*-1e9)                   (DVE/Act)
#   out[s,e] = sum_t p[s,t] x[t,e]                      (PE pass 3)
#
# Precision: plain fp16 single-pass matmuls (fp16 products accumulate exactly
# in fp32 PSUM). End-to-end rel err ~2.3e-3 against the fp32 reference (gate
# 2e-2). The host pre-scales and rounds inputs to fp16 (x,y by 16, W by 1024
# -- exactly the values the kernel would produce on-chip), which halves input
# HBM traffic and removes the entire on-chip scale/cast pipeline; a single DGE
# queue only sustains ~200 GB/s, so inputs ride two HW queues in parallel
# (sync: x+mask+stores, scalar: y+W) directly into persistent slabs.
#
# x and the softmax rows are transposed on the PE (the DMA-xbar transpose path
# ran at ~44 GB/s and starved the output stage). 1/(rowsum*SX) is folded into
# the exp rows before transposing, so the output matmul's PSUM holds the final
# result and the epilogue is a plain fp16 copy + store. Dummy matmuls (HAM
# filler) keep PE utilization up during the DMA-paced phase so the clock
# governor stays at 8/8.
import sys

import numpy as np

for _p in ("/opt/trn_rl_repo",):
    if _p not in sys.path:
        sys.path.insert(0, _p)

import concourse.bass as bass
from concourse import bacc
import concourse.mybir as mybir
import concourse.tile as tile
from concourse.bass_utils import run_bass_kernel_spmd

F32 = mybir.dt.float32
F16 = mybir.dt.float16

P = 128
FD = 512  # matmul moving free dim (one fp32 PSUM bank)

SX = 16.0  # x / y pre-scale (applied on host)
SW = 1024.0  # W pre-scale (applied on host)
SLOG = SX * SX * SW  # net logit scale = 2**18
MASKC = -1.0e9 * SLOG

ALU = mybir.AluOpType
ACTF = mybir.ActivationFunctionType
AXIS = mybir.AxisListType


def build_nc(n=1024, with_bias=False):
    """Build the per-core Bass program (SPMD: same program on all 8 cores)."""
    NT = n // P  # 128-tiles per dim
    NH = n // FD  # 512-halves per dim
    HC = NT // NH  # transposed chunks per half (4)

    nc = bacc.Bacc("TRN2", target_bir_lowering=False, debug=False)
    x_d = nc.dram_tensor("x16", [n, n], F16, kind="ExternalInput")
    xt_d = nc.dram_tensor("xT16", [n, n], F16, kind="ExternalInput")
    y_d = nc.dram_tensor("y16", [n, n], F16, kind="ExternalInput")
    mask_d = nc.dram_tensor("mask16", [n, n], F16, kind="ExternalInput")
    w_d = nc.dram_tensor("W16", [n, n], F16, kind="ExternalInput")
    b_d = None
    if with_bias:
        b_d = nc.dram_tensor("bvec", [1, n], F32, kind="ExternalInput")
    id_d = nc.dram_tensor("ident16", [P, P], F16, kind="ExternalInput")
    out_d = nc.dram_tensor("out", [n, n], F16, kind="ExternalOutput")

    with tile.TileContext(nc) as tc:
        import contextlib

        ctx = contextlib.ExitStack()
        with ctx:
            persist = ctx.enter_context(tc.tile_pool(name="persist", bufs=1))
            epi = ctx.enter_context(tc.tile_pool(name="epi", bufs=4))
            ehp = ctx.enter_context(tc.tile_pool(name="ehp", bufs=4))
            small = ctx.enter_context(tc.tile_pool(name="small", bufs=4))
            psum = ctx.enter_context(tc.tile_pool(name="psum", bufs=3, space="PSUM"))
            psum_r = ctx.enter_context(
                tc.tile_pool(name="psum_r", bufs=1, space="PSUM")
            )
            dram = ctx.enter_context(tc.tile_pool(name="dram", bufs=1, space="DRAM"))

            # ---- persistent slabs (fp16 slabs are [P, NT, n] = 16KB/part) ---
            x_hi = persist.tile([P, NT, n], F16, tag="x_hi")
            y_hi = persist.tile([P, NT, n], F16, tag="y_hi")
            gt_hi = persist.tile([P, NT, n], F16, tag="gt_hi")
            w_hi = persist.tile([P, NT, n], F16, tag="w_hi")
            mask_sl = persist.tile([P, NT, n], F16, tag="mask_sl")
            # transposed x, one slab per s-half: [P, kt, hc, P]
            xTh = [
                persist.tile([P, NT, HC, P], F16, tag=f"xTh_{h}", name=f"xTh_{h}")
                for h in range(NH)
            ]
            ident16 = persist.tile([P, P], F16, tag="ident16")
            scratch = persist.tile([P, FD], F16, tag="scratch")
            rxs = None
            if with_bias:
                rxs = persist.tile([P, NT], F32, tag="rxs")
            recip = [
                persist.tile([P, 1], F32, tag=f"recip{i}", name=f"recip{i}")
                for i in range(NT)
            ]
            et = [
                [
                    persist.tile([P, HC, P], F16, tag=f"et{i}_{h}", name=f"et{i}_{h}")
                    for h in range(NH)
                ]
                for i in range(NT)
            ]
            b_bc = b_sb = None
            if with_bias:
                b_bc = persist.tile([P, n], F32, tag="b_bc")
                b_sb = persist.tile([1, n], F32, tag="b_sb")

            # ident loads first on the sync queue; the PE warmup hangs off it
            nc.sync.dma_start(ident16, id_d[:, :])

            # HAM warm-up: dummy matmuls so the PE clock ramps to 8/8 before
            # the first real transpose/matmul arrives.
            nc.gpsimd.memset(scratch, 0.0)
            wps = psum_r.tile([P, FD], F32, tag="rsx", name="warm_ps")
            for i in range(7):
                nc.tensor.matmul(
                    wps[:, 0:P],
                    lhsT=ident16,
                    rhs=ident16,
                    start=(i == 0),
                    stop=(i == 6),
                )

            def ham_fill(k, name):
                hp = psum_r.tile([P, FD], F32, tag="rsx", name=name)
                for i in range(k):
                    nc.tensor.matmul(
                        hp,
                        lhsT=scratch[:, 0:P],
                        rhs=scratch,
                        start=(i == 0),
                        stop=(i == k - 1),
                    )

            # ---- parallel input streams, striped across both HW DGE queues --
            # Strict need-order: (xT half-0 chunk kt, y kt) pairs for the g
            # wavefront, then xT half-1, W, mask, x rows (out stage only).
            # xTh[h][:, c, hc, pp] = xT16[c*P+p, (h*HC+hc)*P+pp]: one [P, FD]
            # block DMA per (h, c), in the g ladder's consumption order.
            qs = [nc.sync, nc.scalar]

            def xt_load(q, h, c):
                q.dma_start(
                    xTh[h][:, c, :, :],
                    xt_d[P * c : P * (c + 1), FD * h : FD * (h + 1)],
                )

            for kt in range(NT):
                xt_load(qs[kt % 2], 0, kt)
                qs[(kt + 1) % 2].dma_start(
                    y_hi[:, kt, :], y_d[P * kt : P * (kt + 1), :]
                )
            for c in range(NT):
                xt_load(qs[c % 2], 1, c)
            for dt in range(NT):
                qs[dt % 2].dma_start(
                    w_hi[:, dt, :], w_d[P * dt : P * (dt + 1), :]
                )
            for st in range(NT):
                qs[st % 2].dma_start(
                    mask_sl[:, st, :], mask_d[P * st : P * (st + 1), :]
                )
            for it in range(NT):
                qs[it % 2].dma_start(x_hi[:, it, :], x_d[P * it : P * (it + 1), :])

            # bias term applied on DVE in the softmax epilogue:
            #   am += (SLOG/SX * b)[t] * rowsum_x16[s]
            # b broadcast across partitions via a DRAM bounce on the SWDGE
            # queue, which is otherwise idle.
            if with_bias:
                nc.gpsimd.dma_start(b_sb, b_d[:, :])
                nc.vector.tensor_scalar_mul(b_sb, b_sb, SLOG / SX)
                b_dr = dram.tile([1, n], F32, name="b_dr")
                nc.gpsimd.dma_start(b_dr[0:1, :], b_sb)
                bsrc = b_dr[0:1, :]
                nc.gpsimd.dma_start(
                    b_bc[:, :],
                    bass.AP(
                        tensor=bsrc.tensor,
                        offset=bsrc.offset,
                        ap=[[0, P], bsrc.ap[1]],
                    ),
                )


            # g-stage wavefront psums: held open across the whole kt ladder so
            # each arriving y tile immediately unlocks matmuls.
            WFD = list(range(HC))  # dt groups 0..3 of g half 0
            g_ps = {}
            for dt in WFD:
                g_ps[dt] = psum.tile(
                    [P, FD], F32, tag="gwf", bufs=4, name=f"gwf_{dt}"
                )

            def g_wf_step(kt):
                for dt in WFD:
                    nc.tensor.matmul(
                        g_ps[dt],
                        lhsT=y_hi[:, kt, P * dt : P * (dt + 1)],
                        rhs=xTh[0][:, kt, :, :],
                        start=(kt == 0),
                        stop=(kt == NT - 1),
                    )

            for kt in range(NT):
                g_wf_step(kt)
                ham_fill(2, f"hamy_{kt}")

            for dt in WFD:
                nc.vector.tensor_copy(gt_hi[:, dt, 0:FD], g_ps[dt])
            del g_ps

            # ---- rest of g: half 0 dt 4..7, then all of half 1 --------------
            def g_group(sh, dt):
                ps = psum.tile([P, FD], F32, tag="mm", name=f"g{sh}_{dt}")
                for kt in range(NT):
                    nc.tensor.matmul(
                        ps,
                        lhsT=y_hi[:, kt, P * dt : P * (dt + 1)],
                        rhs=xTh[sh][:, kt, :, :],
                        start=(kt == 0),
                        stop=(kt == NT - 1),
                    )
                nc.vector.tensor_copy(gt_hi[:, dt, FD * sh : FD * (sh + 1)], ps)

            for dt in range(HC, NT):
                g_group(0, dt)
            for dt in range(NT):
                g_group(1, dt)

            # ---- a stage + softmax, with eT transposes pipelined ------------
            eh_l = [None] * NT
            rs_l = [None] * NT

            def a_emit(st):
                if with_bias:
                    # bias rowsum from the x rows, paced one tile per a-tile
                    # so it never head-of-line blocks the DVE queue
                    nc.vector.tensor_reduce(
                        rxs[:, st : st + 1],
                        x_hi[:, st, :],
                        axis=AXIS.X,
                        op=ALU.add,
                    )
                am = epi.tile([P, n], F32, tag="am", name=f"am{st}")
                for th in range(NH):
                    ps = psum.tile([P, FD], F32, tag="mm", name=f"a{st}_{th}")
                    for dt in range(NT):
                        nc.tensor.matmul(
                            ps,
                            lhsT=gt_hi[:, dt, P * st : P * (st + 1)],
                            rhs=w_hi[:, dt, FD * th : FD * (th + 1)],
                            start=(dt == 0),
                            stop=(dt == NT - 1),
                        )
                    # masked scaled logits: am = mask*MASKC + psum
                    nc.vector.scalar_tensor_tensor(
                        out=am[:, FD * th : FD * (th + 1)],
                        in0=mask_sl[:, st, FD * th : FD * (th + 1)],
                        scalar=MASKC,
                        in1=ps,
                        op0=ALU.mult,
                        op1=ALU.add,
                    )
                if with_bias:
                    # am += b_bc[t] * rowsum_x16[s]  (rank-1 bias)
                    nc.vector.scalar_tensor_tensor(
                        out=am,
                        in0=b_bc,
                        scalar=rxs[:, st : st + 1],
                        in1=am,
                        op0=ALU.mult,
                        op1=ALU.add,
                    )
                nm = small.tile([P, 1], F32, tag="nm", name=f"nm{st}")
                nc.vector.tensor_reduce(
                    nm, am, axis=AXIS.X, op=ALU.max, negate=True
                )
                nms = small.tile([P, 1], F32, tag="nms", name=f"nms{st}")
                nc.vector.tensor_scalar_mul(nms, nm, 1.0 / SLOG)
                eh = ehp.tile([P, n], F16, tag="eh", name=f"eh{st}")
                rs = small.tile([P, 1], F32, tag="rs", name=f"rs{st}")
                nc.scalar.activation(
                    eh, am, ACTF.Exp, bias=nms, scale=1.0 / SLOG, accum_out=rs
                )
                eh_l[st] = eh
                rs_l[st] = rs

            def finish_emit(st):
                # fold 1/(rowsum * SX) into the exp rows so the out-stage psum
                # holds the final output directly
                rs2 = small.tile([P, 1], F32, tag="rs2", name=f"rs2_{st}")
                nc.vector.tensor_scalar_mul(rs2, rs_l[st], SX)
                nc.vector.reciprocal(recip[st], rs2)
                nc.vector.tensor_scalar_mul(eh_l[st], eh_l[st], recip[st])

            def t_emit(st):
                # transpose the scaled exp rows on the PE (8 chunks of 128x128)
                for h in range(NH):
                    pt = psum.tile(
                        [P, HC, P], F16, tag="gwf", bufs=4, name=f"eT{st}_{h}"
                    )
                    for j in range(HC):
                        c = h * HC + j
                        nc.tensor.transpose(
                            pt[:, j, :],
                            eh_l[st][:, P * c : P * (c + 1)],
                            ident16,
                        )
                    nc.vector.tensor_copy(et[st][h], pt)

            def out_emit(st):
                # both halves interleaved: consecutive matmuls share lhsT
                opair = []
                for h in range(NH):
                    ps = psum.tile([P, FD], F32, tag="mm", name=f"o{st}_{h}")
                    for tt in range(NT):
                        nc.tensor.matmul(
                            ps,
                            lhsT=et[st][tt // HC][:, tt % HC, :],
                            rhs=x_hi[:, tt, FD * h : FD * (h + 1)],
                            start=(tt == 0),
                            stop=(tt == NT - 1),
                        )
                    opair.append((h, ps))
                for h, ps in opair:
                    ob = epi.tile([P, FD], F16, tag="ob", name=f"ob{st}_{h}")
                    if h == 0:
                        nc.scalar.copy(ob, ps)
                        nc.sync.dma_start(
                            out_d[P * st : P * (st + 1), 0:FD], ob
                        )
                    else:
                        nc.vector.tensor_copy(ob, ps)
                        nc.scalar.dma_start(
                            out_d[P * st : P * (st + 1), FD:n], ob
                        )

            for st in range(NT):
                a_emit(st)
                if st >= 2:
                    finish_emit(st - 2)
                    t_emit(st - 2)
            finish_emit(NT - 2)
            t_emit(NT - 2)
            out_emit(0)
            finish_emit(NT - 1)
            t_emit(NT - 1)
            for st in range(1, NT):
                out_emit(st)
    nc.compile()
    return nc


_NC_CACHE = {}


def _get_nc(n=1024, with_bias=False):
    key = (n, with_bias)
    if key not in _NC_CACHE:
        _NC_CACHE[key] = build_nc(n, with_bias)
    return _NC_CACHE[key]


def _prep_in_maps(x, y, mask, W, b, with_bias=False):
    """Host-side shard prep: pre-scaled fp16 copies of the inputs."""
    n = x.shape[-1]
    x16 = (np.asarray(x, dtype=np.float32) * SX).astype(np.float16)
    y16 = (np.asarray(y, dtype=np.float32) * SX).astype(np.float16)
    m16 = np.asarray(mask, dtype=np.float32).astype(np.float16)
    W16 = np.ascontiguousarray(
        (np.asarray(W, dtype=np.float32) * SW).astype(np.float16)
    )
    bc = np.ascontiguousarray(np.asarray(b, dtype=np.float32).reshape(1, n))
    idc = np.eye(P, dtype=np.float16)
    in_maps = []
    for c in range(x.shape[0]):
        in_maps.append(
            {
                "x16": np.ascontiguousarray(x16[c]),
                "xT16": np.ascontiguousarray(x16[c].T),
                "y16": np.ascontiguousarray(y16[c]),
                "mask16": np.ascontiguousarray(m16[c]),
                "W16": W16,
                "ident16": idc,
            }
        )
        if with_bias:
            in_maps[-1]["bvec"] = bc
    return in_maps


def kernel(x, y, mask, W, b):
    """Full-input entry point: shard over batch across 8 cores, run, gather."""
    n = x.shape[-1]
    with_bias = bool(np.any(np.asarray(b)))
    nc = _get_nc(n, with_bias)
    in_maps = _prep_in_maps(x, y, mask, W, b, with_bias)
    res = run_bass_kernel_spmd(nc, in_maps, core_ids=list(range(len(in_maps))))
    return np.stack([r["out"] for r in res.results], axis=0).astype(np.float32)
